# revision 24
# baseline (speedup 1.0000x reference)
"""Trainium2 Bass kernel for nn_EnrichmentLayer (sparse block-diagonal attention).

Key insight: the reference multiplies the +-1e9 mask into the scores before
softmax. For every (head, row) the resulting softmax is EXACTLY one-hot at the
out-of-graph key with the most negative score (verified numerically: max
weight == 1.0 for all 65536 rows). So attention reduces to
    out[n,h,:] = v[argmin_{m: batch[n] != sub_batch[m]} (q_n . k_m), h, :]
which we compute with a fused masked row-max (tensor_mask_reduce), an
argmax-index extraction, and an indirect-DMA gather of v rows.

Sharding: query rows (N=4096) split across 8 cores (512 rows each); k/v
computed replicated on every core from the full sh.
"""

import numpy as np

N = 4096
M = 4096
D = 128
HEADS = 16
HD = 8
NCORES = 8
ROWS = N // NCORES          # query rows per core = 512
QC = ROWS // 128            # 128-row chunks per core = 4
EPS = 1e-5
NEG_BIG = -3.0e38

_CACHED = {}
_last_in_maps = None


def _build_program(apply_affine, gather_mode="indirect"):
    """Build the (single, shared across cores) Bass program."""
    import concourse.bass as bass
    import concourse.bacc as bacc_mod
    import concourse.mybir as mybir
    import concourse.tile as tile
    from concourse.dve_ops import TENSOR_MASK_REDUCE, TENSOR_TENSOR_REDUCE

    fp32 = mybir.dt.float32
    fp16 = mybir.dt.float16
    int32 = mybir.dt.int32
    AL = mybir.AluOpType
    AF = mybir.ActivationFunctionType
    AX = mybir.AxisListType

    nc = bacc_mod.Bacc()

    # ---------------- DRAM I/O ----------------
    def din(name, shape):
        return nc.dram_tensor(name, shape, fp32, kind="ExternalInput")

    h_own = din("h_own", [ROWS, D])
    sh = din("sh", [M, D])
    W = {}
    for w in ["wq1", "wq2", "wk1", "wk2", "wv1", "wv2", "wo1a", "wo1b", "wo2"]:
        W[w] = din(w, [D, D])
    B = {}
    for b in ["bq1", "bq2", "bk1", "bk2", "bv1", "bv2", "bo1"]:
        B[b] = din(b, [D, 1])
    G = {}
    if apply_affine:
        for g in ["gq_b", "beq_b", "gk_b", "bek_b", "gv_b", "bev_b", "go_b", "beo_b"]:
            G[g] = din(g, [D, D])
    bo2_b = din("bo2_b", [D, D])
    ident_d = din("ident", [D, D])
    msA = din("msA", [D, QC])
    meA = din("meA", [D, QC])
    msB = din("msB", [D, QC])
    meB = din("meB", [D, QC])
    iota_d = din("iota_b", [D, M])

    out_d = nc.dram_tensor("out", [ROWS, D], fp32, kind="ExternalOutput")

    # v rows per head in DRAM for the gather (indirect src offset must be 0
    # => one tensor per head)
    v_dram = [nc.dram_tensor(f"vd{h}", [M, HD], fp32, kind="Internal")
              for h in range(HEADS)]

    with tile.TileContext(nc) as tc, \
         tc.tile_pool(name="persist", bufs=1) as pp:

        # persistent SBUF tensors
        ident = pp.tile([128, 128], fp32, tag="ident")
        nc.sync.dma_start(ident[:], ident_d[:])
        eps_t = pp.tile([128, 1], fp32, tag="eps")
        nc.gpsimd.memset(eps_t[:], EPS)

        shT = pp.tile([128, M], fp32, tag="shT")        # sh transposed [feat, rows]
        hT = pp.tile([128, ROWS], fp32, tag="hT")       # h_own transposed
        h_nat = pp.tile([128, QC * 128], fp32, tag="h_nat")  # h_own natural (per qc slices)
        kT = pp.tile([128, M], fp32, tag="kT")
        qT = pp.tile([128, ROWS], fp32, tag="qT")
        iota_b = pp.tile([128, M], fp32, tag="iota")
        nc.sync.dma_start(iota_b[:], iota_d[:])

        wt = {}
        for w in W:
            wt[w] = pp.tile([128, 128], fp32, tag=f"w_{w}", name=f"w_{w}")
            nc.sync.dma_start(wt[w][:], W[w][:])
        bt = {}
        for b in B:
            bt[b] = pp.tile([128, 1], fp32, tag=f"b_{b}", name=f"b_{b}")
            nc.sync.dma_start(bt[b][:], B[b][:])
        gt = {}
        for g in G:
            gt[g] = pp.tile([128, 128], fp32, tag=f"g_{g}", name=f"g_{g}")
            nc.sync.dma_start(gt[g][:], G[g][:])
        mst = {}
        for mname, md in [("msA", msA), ("meA", meA), ("msB", msB), ("meB", meB)]:
            mst[mname] = pp.tile([128, QC], fp32, tag=f"m_{mname}", name=f"m_{mname}")
            nc.sync.dma_start(mst[mname][:], md[:])
        bo2_t = pp.tile([128, 128], fp32, tag="bo2b")
        nc.sync.dma_start(bo2_t[:], bo2_b[:])

        # ---------------- load + transpose inputs ----------------
        with tc.tile_pool(name="ld_sb", bufs=3) as lsb, \
             tc.tile_pool(name="ld_ps", bufs=3, space="PSUM") as lps:
            for c in range(M // 128):
                t = lsb.tile([128, 128], fp32, tag="ldt")
                nc.sync.dma_start(t[:], sh[c * 128:(c + 1) * 128, :])
                ps = lps.tile([128, 128], fp32, tag="ldp")
                nc.tensor.transpose(ps[:], t[:], ident[:])
                nc.scalar.copy(shT[:, c * 128:(c + 1) * 128], ps[:])
            for c in range(QC):
                t = lsb.tile([128, 128], fp32, tag="ldt")
                nc.sync.dma_start(t[:], h_own[c * 128:(c + 1) * 128, :])
                nc.vector.tensor_copy(h_nat[:, c * 128:(c + 1) * 128], t[:])
                ps = lps.tile([128, 128], fp32, tag="ldp")
                nc.tensor.transpose(ps[:], t[:], ident[:])
                nc.scalar.copy(hT[:, c * 128:(c + 1) * 128], ps[:])

        # ---------------- MLPs (transposed activations, LN via transpose sandwich) ----------------
        def mlp_T(xT_ap, rows, w1, b1, g_b, be_b, w2, b2, outT_ap, mm_pool, tr_pool, sb_pool):
            """outT = mlp(x)^T with x given as xT [feat, rows]. All f32."""
            nchunks = rows // 512
            for c in range(nchunks):
                sl = slice(c * 512, (c + 1) * 512)
                y1p = mm_pool.tile([128, 512], fp32, tag="y1p")
                nc.tensor.matmul(y1p[:], lhsT=w1[:], rhs=xT_ap[:, sl], start=True, stop=True)
                y1s = sb_pool.tile([128, 512], fp32, tag="y1s")
                nc.vector.tensor_scalar(out=y1s[:], in0=y1p[:], scalar1=b1[:],
                                        scalar2=None, op0=AL.add)
                # transpose to natural [rows, hid]
                nat = sb_pool.tile([128, 512], fp32, tag="nat")
                for s in range(4):
                    ssl = slice(s * 128, (s + 1) * 128)
                    np_ = tr_pool.tile([128, 128], fp32, tag="trp")
                    nc.tensor.transpose(np_[:], y1s[:, ssl], ident[:])
                    nc.scalar.copy(nat[:, ssl], np_[:])
                # LN stats over hid (free dim), batched for the 4 sub-tiles
                sums = sb_pool.tile([128, 4], fp32, tag="sums")
                nc.vector.tensor_reduce(out=sums[:], in_=nat[:].rearrange("p (s f) -> p s f", f=128),
                                        axis=AX.X, op=AL.add)
                sq = sb_pool.tile([128, 512], fp32, tag="sq")
                nc.vector.tensor_tensor(out=sq[:], in0=nat[:], in1=nat[:], op=AL.mult)
                sums2 = sb_pool.tile([128, 4], fp32, tag="sums2")
                nc.vector.tensor_reduce(out=sums2[:], in_=sq[:].rearrange("p (s f) -> p s f", f=128),
                                        axis=AX.X, op=AL.add)
                mu = sb_pool.tile([128, 4], fp32, tag="mu")
                nc.vector.tensor_scalar(out=mu[:], in0=sums[:], scalar1=1.0 / 128, scalar2=None, op0=AL.mult)
                ex2 = sb_pool.tile([128, 4], fp32, tag="ex2")
                nc.vector.tensor_scalar(out=ex2[:], in0=sums2[:], scalar1=1.0 / 128, scalar2=None, op0=AL.mult)
                mu2 = sb_pool.tile([128, 4], fp32, tag="mu2")
                nc.vector.tensor_tensor(out=mu2[:], in0=mu[:], in1=mu[:], op=AL.mult)
                var = sb_pool.tile([128, 4], fp32, tag="var")
                nc.vector.tensor_tensor(out=var[:], in0=ex2[:], in1=mu2[:], op=AL.subtract)
                sd = sb_pool.tile([128, 4], fp32, tag="sd")
                nc.scalar.activation(sd[:], var[:], AF.Sqrt, bias=eps_t[:], scale=1.0)
                rstd = sb_pool.tile([128, 4], fp32, tag="rstd")
                nc.vector.reciprocal(rstd[:], sd[:])
                # normalize (+ affine) + relu, then transpose back
                nrm = sb_pool.tile([128, 512], fp32, tag="nrm")
                for s in range(4):
                    ssl = slice(s * 128, (s + 1) * 128)
                    nc.vector.tensor_scalar(out=nrm[:, ssl], in0=nat[:, ssl],
                                            scalar1=mu[:, s:s + 1], scalar2=rstd[:, s:s + 1],
                                            op0=AL.subtract, op1=AL.mult)
                if g_b is not None:
                    nc.vector.tensor_tensor(out=nrm[:], in0=nrm[:], in1=g_b[:].to_broadcast([128, 512]), op=AL.mult)
                    nc.vector.tensor_tensor(out=nrm[:], in0=nrm[:], in1=be_b[:].to_broadcast([128, 512]), op=AL.add)
                rl = sb_pool.tile([128, 512], fp32, tag="rl")
                nc.vector.tensor_scalar(out=rl[:], in0=nrm[:], scalar1=0.0, scalar2=None, op0=AL.max)
                yTr = sb_pool.tile([128, 512], fp32, tag="yTr")
                for s in range(4):
                    ssl = slice(s * 128, (s + 1) * 128)
                    np2 = tr_pool.tile([128, 128], fp32, tag="trp")
                    nc.tensor.transpose(np2[:], rl[:, ssl], ident[:])
                    nc.scalar.copy(yTr[:, ssl], np2[:])
                y2p = mm_pool.tile([128, 512], fp32, tag="y2p")
                nc.tensor.matmul(y2p[:], lhsT=w2[:], rhs=yTr[:], start=True, stop=True)
                nc.vector.tensor_scalar(out=outT_ap[:, sl], in0=y2p[:], scalar1=b2[:],
                                        scalar2=None, op0=AL.add)

        ga = (lambda k_: gt[k_] if apply_affine else None)
        with tc.tile_pool(name="mlp_mm", bufs=2, space="PSUM") as mmp, \
             tc.tile_pool(name="mlp_tr", bufs=4, space="PSUM") as trp, \
             tc.tile_pool(name="mlp_sb", bufs=2) as msb:
            mlp_T(qT_in := hT, ROWS, wt["wq1"], bt["bq1"], ga("gq_b"),
                  ga("beq_b") if apply_affine else None, wt["wq2"], bt["bq2"], qT, mmp, trp, msb)
            mlp_T(shT, M, wt["wk1"], bt["bk1"], ga("gk_b"),
                  ga("bek_b") if apply_affine else None, wt["wk2"], bt["bk2"], kT, mmp, trp, msb)
            # v: compute transposed, then transpose to natural and store per-head to DRAM
            vT = pp.tile([128, M], fp32, tag="vT")
            mlp_T(shT, M, wt["wv1"], bt["bv1"], ga("gv_b"),
                  ga("bev_b") if apply_affine else None, wt["wv2"], bt["bv2"], vT, mmp, trp, msb)
            for c in range(M // 128):
                vp = trp.tile([128, 128], fp32, tag="trp")
                nc.tensor.transpose(vp[:], vT[:, c * 128:(c + 1) * 128], ident[:])
                vn = msb.tile([128, 128], fp32, tag="vns")
                nc.scalar.copy(vn[:], vp[:])
                for h in range(HEADS):
                    nc.sync.dma_start(v_dram[h][c * 128:(c + 1) * 128, :],
                                      vn[:, h * HD:(h + 1) * HD])

        # ---------------- fp16 hi/lo splits of qT, kT ----------------
        kh16 = pp.tile([128, M], fp16, tag="kh16")
        kl16 = pp.tile([128, M], fp16, tag="kl16")
        qh16 = pp.tile([128, ROWS], fp16, tag="qh16")
        ql16 = pp.tile([128, ROWS], fp16, tag="ql16")
        with tc.tile_pool(name="split_sb", bufs=2) as ssb:
            for nm, src_t, hi, lo, width in (("k", kT, kh16, kl16, M),
                                             ("q", qT, qh16, ql16, ROWS)):
                nc.scalar.copy(hi[:], src_t[:])
                hf = ssb.tile([128, width], fp32, tag="hf", name=f"hf_{nm}")
                nc.scalar.copy(hf[:], hi[:])
                lr = ssb.tile([128, width], fp32, tag="lr", name=f"lr_{nm}")
                nc.vector.tensor_tensor(out=lr[:], in0=src_t[:], in1=hf[:], op=AL.subtract)
                nc.scalar.copy(lo[:], lr[:])

        # ---------------- attention: scores + masked argmin + gather ----------------
        attn = pp.tile([128, QC * 128], fp32, tag="attn")   # gathered v rows, natural layout
        with tc.tile_pool(name="z_ps", bufs=2, space="PSUM") as zp, \
             tc.tile_pool(name="att_sb", bufs=1) as asb, \
             tc.tile_pool(name="att_small", bufs=4) as ats:
            for h in range(HEADS):
                hsl = slice(h * HD, (h + 1) * HD)
                # stage this head's fp16 pieces at partition base 0:
                # rows [0:8]=hi, [8:16]=hi(q)/lo(k), [16:24]=lo(q)/hi(k)
                # pairing: qh*kh + qh*kl + ql*kh
                q_st = ats.tile([3 * HD, ROWS], fp16, tag="q_st")
                nc.sync.dma_start(q_st[0 * HD:1 * HD, :], qh16[hsl, :])
                nc.sync.dma_start(q_st[1 * HD:2 * HD, :], qh16[hsl, :])
                nc.sync.dma_start(q_st[2 * HD:3 * HD, :], ql16[hsl, :])
                k_st = asb.tile([3 * HD, M], fp16, tag="k_st")
                nc.sync.dma_start(k_st[0 * HD:1 * HD, :], kh16[hsl, :])
                nc.sync.dma_start(k_st[1 * HD:2 * HD, :], kl16[hsl, :])
                nc.sync.dma_start(k_st[2 * HD:3 * HD, :], kh16[hsl, :])
                for qc in range(QC):
                    qsl = slice(qc * 128, (qc + 1) * 128)
                    Wm = asb.tile([128, M], fp32, tag="Wm")
                    racc = None
                    for half in range(2):
                        zt = zp.tile([128, 2048], fp32, tag="zt")
                        for j in range(4):
                            col0 = half * 2048 + j * 512
                            nc.tensor.matmul(zt[:, j * 512:(j + 1) * 512],
                                             lhsT=q_st[:, qsl],
                                             rhs=k_st[:, col0:col0 + 512],
                                             start=True, stop=True)
                        rnew = ats.tile([128, 1], fp32, tag="racc")
                        ms = mst["msA" if half == 0 else "msB"]
                        me = mst["meA" if half == 0 else "meB"]
                        nc.vector._custom_dve(
                            TENSOR_MASK_REDUCE,
                            out=Wm[:, half * 2048:(half + 1) * 2048], in0=zt[:],
                            in1=me[:, qc:qc + 1],
                            s0=ms[:, qc:qc + 1],
                            s1=(NEG_BIG if racc is None else racc[:]),
                            imm2=1.0, accum_out=rnew[:])
                        racc = rnew
                    # index of the (negated-score) maximum
                    E = asb.tile([128, M], fp32, tag="E")
                    nc.vector.tensor_scalar(out=E[:], in0=Wm[:], scalar1=racc[:],
                                            scalar2=None, op0=AL.is_ge)
                    idxf = ats.tile([128, 1], fp32, tag="idxf")
                    nc.vector._custom_dve(TENSOR_TENSOR_REDUCE,
                                          out=E[:], in0=E[:], in1=iota_b[:],
                                          s0=0.0, s1=1.0, accum_out=idxf[:])
                    idxi = ats.tile([128, 1], int32, tag="idxi")
                    nc.vector.tensor_copy(idxi[:], idxf[:])
                    if gather_mode == "indirect":
                        nc.gpsimd.indirect_dma_start(
                            out=attn[:, qc * 128 + h * HD: qc * 128 + (h + 1) * HD],
                            out_offset=None,
                            in_=v_dram[h][:],
                            in_offset=bass.IndirectOffsetOnAxis(ap=idxi[:, :1], axis=0),
                        )
                    else:
                        nc.sync.dma_start(
                            out=attn[:, qc * 128 + h * HD: qc * 128 + (h + 1) * HD],
                            in_=v_dram[h][0:128, :])

        # ---------------- output MLP + residual ----------------
        with tc.tile_pool(name="o_mm", bufs=2, space="PSUM") as omp, \
             tc.tile_pool(name="o_tr", bufs=4, space="PSUM") as otp, \
             tc.tile_pool(name="o_sb", bufs=3) as osb:
            # attn^T
            attnT = pp.tile([128, ROWS], fp32, tag="attnT")
            for c in range(QC):
                ap_ = otp.tile([128, 128], fp32, tag="atp")
                nc.tensor.transpose(ap_[:], attn[:, c * 128:(c + 1) * 128], ident[:])
                nc.scalar.copy(attnT[:, c * 128:(c + 1) * 128], ap_[:])
            # layer1: y1 = cat(attn, h) @ Wo1 (+bo1)  in transposed layout
            y1p = omp.tile([128, 512], fp32, tag="oy1p")
            nc.tensor.matmul(y1p[:], lhsT=wt["wo1a"][:], rhs=attnT[:], start=True, stop=False)
            nc.tensor.matmul(y1p[:], lhsT=wt["wo1b"][:], rhs=hT[:], start=False, stop=True)
            y1s = osb.tile([128, 512], fp32, tag="oy1s")
            nc.vector.tensor_scalar(out=y1s[:], in0=y1p[:], scalar1=bt["bo1"][:],
                                    scalar2=None, op0=AL.add)
            nat = osb.tile([128, 512], fp32, tag="onat")
            for s in range(4):
                ssl = slice(s * 128, (s + 1) * 128)
                np_ = otp.tile([128, 128], fp32, tag="atp")
                nc.tensor.transpose(np_[:], y1s[:, ssl], ident[:])
                nc.scalar.copy(nat[:, ssl], np_[:])
            sums = osb.tile([128, 4], fp32, tag="osums")
            nc.vector.tensor_reduce(out=sums[:], in_=nat[:].rearrange("p (s f) -> p s f", f=128),
                                    axis=AX.X, op=AL.add)
            sq = osb.tile([128, 512], fp32, tag="osq")
            nc.vector.tensor_tensor(out=sq[:], in0=nat[:], in1=nat[:], op=AL.mult)
            sums2 = osb.tile([128, 4], fp32, tag="osums2")
            nc.vector.tensor_reduce(out=sums2[:], in_=sq[:].rearrange("p (s f) -> p s f", f=128),
                                    axis=AX.X, op=AL.add)
            mu = osb.tile([128, 4], fp32, tag="omu")
            nc.vector.tensor_scalar(out=mu[:], in0=sums[:], scalar1=1.0 / 128, scalar2=None, op0=AL.mult)
            ex2 = osb.tile([128, 4], fp32, tag="oex2")
            nc.vector.tensor_scalar(out=ex2[:], in0=sums2[:], scalar1=1.0 / 128, scalar2=None, op0=AL.mult)
            mu2 = osb.tile([128, 4], fp32, tag="omu2")
            nc.vector.tensor_tensor(out=mu2[:], in0=mu[:], in1=mu[:], op=AL.mult)
            var = osb.tile([128, 4], fp32, tag="ovar")
            nc.vector.tensor_tensor(out=var[:], in0=ex2[:], in1=mu2[:], op=AL.subtract)
            sd = osb.tile([128, 4], fp32, tag="osd")
            nc.scalar.activation(sd[:], var[:], AF.Sqrt, bias=eps_t[:], scale=1.0)
            rstd = osb.tile([128, 4], fp32, tag="orstd")
            nc.vector.reciprocal(rstd[:], sd[:])
            nrm = osb.tile([128, 512], fp32, tag="onrm")
            for s in range(4):
                ssl = slice(s * 128, (s + 1) * 128)
                nc.vector.tensor_scalar(out=nrm[:, ssl], in0=nat[:, ssl],
                                        scalar1=mu[:, s:s + 1], scalar2=rstd[:, s:s + 1],
                                        op0=AL.subtract, op1=AL.mult)
            if apply_affine:
                nc.vector.tensor_tensor(out=nrm[:], in0=nrm[:], in1=gt["go_b"][:].to_broadcast([128, 512]), op=AL.mult)
                nc.vector.tensor_tensor(out=nrm[:], in0=nrm[:], in1=gt["beo_b"][:].to_broadcast([128, 512]), op=AL.add)
            rl = osb.tile([128, 512], fp32, tag="orl")
            nc.vector.tensor_scalar(out=rl[:], in0=nrm[:], scalar1=0.0, scalar2=None, op0=AL.max)
            # layer2 in natural layout per qc chunk + bias + residual
            for s in range(4):
                ssl = slice(s * 128, (s + 1) * 128)
                yTr = osb.tile([128, 128], fp32, tag="oyTr")
                np2 = otp.tile([128, 128], fp32, tag="atp")
                nc.tensor.transpose(np2[:], rl[:, ssl], ident[:])
                nc.scalar.copy(yTr[:], np2[:])
                y2p = omp.tile([128, 128], fp32, tag="oy2p")
                nc.tensor.matmul(y2p[:], lhsT=yTr[:], rhs=wt["wo2"][:], start=True, stop=True)
                fin = osb.tile([128, 128], fp32, tag="fin")
                # + bo2 (replicated across partitions on host; free dim = out feature)
                nc.vector.tensor_tensor(out=fin[:], in0=y2p[:], in1=bo2_t[:], op=AL.add)
                nc.vector.tensor_tensor(out=fin[:], in0=fin[:], in1=h_nat[:, ssl], op=AL.add)
                nc.sync.dma_start(out_d[s * 128:(s + 1) * 128, :], fin[:])

    nc.compile()
    return nc


def _window_encoding(batch, sub_batch):
    """Per-row (mask_start, mask_end) for the two 2048-wide halves."""
    b = np.asarray(batch).astype(np.int64)
    sb = np.asarray(sub_batch).astype(np.int64)
    a = np.searchsorted(sb, b, side="left").astype(np.int64)
    e = np.searchsorted(sb, b, side="right").astype(np.int64)
    enc = []
    for off in (0, 2048):
        wa = np.clip(a - off, 0, 2048)
        wb = np.clip(e - off, 0, 2048)
        ms = np.where(wa == wb, 0.0, wb.astype(np.float64))
        me = np.where(wa == wb, 2048.0, wa.astype(np.float64))
        enc.append((ms.astype(np.float32), me.astype(np.float32)))
    return enc  # [(msA, meA), (msB, meB)] each [N]


def kernel(h, sh, batch, sub_batch, params):
    from concourse.bass_utils import run_bass_kernel_spmd

    h = np.ascontiguousarray(np.asarray(h, dtype=np.float32))
    sh = np.ascontiguousarray(np.asarray(sh, dtype=np.float32))
    p = params

    def col(x):
        return np.ascontiguousarray(np.asarray(x, dtype=np.float32).reshape(D, 1))

    def matf(x):
        return np.ascontiguousarray(np.asarray(x, dtype=np.float32))

    affine_identity = all(
        np.all(np.asarray(p[k]["b1"]) == 0) and np.all(np.asarray(p[k]["g"]) == 1)
        and np.all(np.asarray(p[k]["be"]) == 0)
        for k in ("hk", "hv", "hq", "out")
    )
    apply_affine = not affine_identity

    import os
    gather_mode = os.environ.get("GATHER_MODE", "indirect")
    key = ("prog", apply_affine, gather_mode)
    if key not in _CACHED:
        _CACHED[key] = _build_program(apply_affine, gather_mode)
    nc = _CACHED[key]

    (msA, meA), (msB, meB) = _window_encoding(batch, sub_batch)
    iota_b = np.ascontiguousarray(
        np.broadcast_to(np.arange(M, dtype=np.float32), (D, M)))

    base = {
        "sh": sh,
        "wq1": matf(p["hq"]["w1"]), "wq2": matf(-np.asarray(p["hq"]["w2"], dtype=np.float32)),
        "wk1": matf(p["hk"]["w1"]), "wk2": matf(p["hk"]["w2"]),
        "wv1": matf(p["hv"]["w1"]), "wv2": matf(p["hv"]["w2"]),
        "wo1a": matf(np.asarray(p["out"]["w1"], dtype=np.float32)[:D, :]),
        "wo1b": matf(np.asarray(p["out"]["w1"], dtype=np.float32)[D:, :]),
        "wo2": matf(p["out"]["w2"]),
        "bq1": col(p["hq"]["b1"]), "bq2": col(-np.asarray(p["hq"]["b2"], dtype=np.float32)),
        "bk1": col(p["hk"]["b1"]), "bk2": col(p["hk"]["b2"]),
        "bv1": col(p["hv"]["b1"]), "bv2": col(p["hv"]["b2"]),
        "bo1": col(p["out"]["b1"]),
        "bo2_b": np.ascontiguousarray(np.broadcast_to(
            np.asarray(p["out"]["b2"], dtype=np.float32)[None, :], (D, D))),
        "iota_b": iota_b,
        "ident": np.eye(D, dtype=np.float32),
    }
    if apply_affine:
        def bcast(x):
            return np.ascontiguousarray(
                np.broadcast_to(np.asarray(x, dtype=np.float32)[None, :], (D, D)))
        base.update({
            "gq_b": bcast(p["hq"]["g"]), "beq_b": bcast(p["hq"]["be"]),
            "gk_b": bcast(p["hk"]["g"]), "bek_b": bcast(p["hk"]["be"]),
            "gv_b": bcast(p["hv"]["g"]), "bev_b": bcast(p["hv"]["be"]),
            "go_b": bcast(p["out"]["g"]), "beo_b": bcast(p["out"]["be"]),
        })

    def chunkify(x, c):
        # [512] row-vector -> [128, QC] column-per-qc layout
        return np.ascontiguousarray(
            x[c * ROWS:(c + 1) * ROWS].reshape(QC, 128).T.astype(np.float32))

    in_maps = []
    for c in range(NCORES):
        m = dict(base)
        m["h_own"] = np.ascontiguousarray(h[c * ROWS:(c + 1) * ROWS, :])
        m["msA"] = chunkify(msA, c)
        m["meA"] = chunkify(meA, c)
        m["msB"] = chunkify(msB, c)
        m["meB"] = chunkify(meB, c)
        in_maps.append(m)

    global _last_in_maps
    _last_in_maps = in_maps
    res = run_bass_kernel_spmd(nc, in_maps, core_ids=list(range(NCORES)))
    out = np.concatenate([res.results[c]["out"] for c in range(NCORES)], axis=0)
    return out.astype(np.float32)


if __name__ == "__main__":
    import reference
    inp = reference.setup_inputs()
    exp = np.asarray(reference.reference(**inp))
    act = kernel(**{k: (v if isinstance(v, dict) else np.asarray(v)) for k, v in inp.items()})
    err = np.linalg.norm(act - exp) / np.linalg.norm(exp)
    print("Relative error:", err)


# revision 26
# speedup vs baseline: 1.0875x; 1.0875x over previous
"""Trainium2 Bass kernel for nn_EnrichmentLayer (sparse block-diagonal attention).

Key insight: the reference multiplies the +-1e9 mask into the scores before
softmax. For every (head, row) the resulting softmax is EXACTLY one-hot at the
out-of-graph key with the most negative score (verified numerically: max
weight == 1.0 for all 65536 rows). So attention reduces to
    out[n,h,:] = v[argmin_{m: batch[n] != sub_batch[m]} (q_n . k_m), h, :]
which we compute with a fused masked row-max (tensor_mask_reduce), an
argmax-index extraction, and an indirect-DMA gather of v rows.

Sharding: query rows (N=4096) split across 8 cores (512 rows each); k/v
computed replicated on every core from the full sh.
"""

import numpy as np

N = 4096
M = 4096
D = 128
HEADS = 16
HD = 8
NCORES = 8
ROWS = N // NCORES          # query rows per core = 512
QC = ROWS // 128            # 128-row chunks per core = 4
EPS = 1e-5
NEG_BIG = -3.0e38

_CACHED = {}
_last_in_maps = None


def _build_program(apply_affine, gather_mode="indirect"):
    """Build the (single, shared across cores) Bass program."""
    import concourse.bass as bass
    import concourse.bacc as bacc_mod
    import concourse.mybir as mybir
    import concourse.tile as tile
    from concourse.dve_ops import TENSOR_MASK_REDUCE, TENSOR_TENSOR_REDUCE

    fp32 = mybir.dt.float32
    fp16 = mybir.dt.float16
    int32 = mybir.dt.int32
    AL = mybir.AluOpType
    AF = mybir.ActivationFunctionType
    AX = mybir.AxisListType

    nc = bacc_mod.Bacc()

    # ---------------- DRAM I/O ----------------
    def din(name, shape):
        return nc.dram_tensor(name, shape, fp32, kind="ExternalInput")

    h_own = din("h_own", [ROWS, D])
    sh = din("sh", [M, D])
    W = {}
    for w in ["wq1", "wq2", "wk1", "wk2", "wv1", "wv2", "wo1a", "wo1b", "wo2"]:
        W[w] = din(w, [D, D])
    B = {}
    for b in ["bq1", "bq2", "bk1", "bk2", "bv1", "bv2", "bo1"]:
        B[b] = din(b, [D, 1])
    G = {}
    if apply_affine:
        for g in ["gq_b", "beq_b", "gk_b", "bek_b", "gv_b", "bev_b", "go_b", "beo_b"]:
            G[g] = din(g, [D, D])
    bo2_b = din("bo2_b", [D, D])
    ident_d = din("ident", [D, D])
    msA = din("msA", [D, QC])
    meA = din("meA", [D, QC])
    msB = din("msB", [D, QC])
    meB = din("meB", [D, QC])
    iota_d = din("iota_b", [D, M])

    out_d = nc.dram_tensor("out", [ROWS, D], fp32, kind="ExternalOutput")

    # v rows per head in DRAM for the gather (indirect src offset must be 0
    # => one tensor per head)
    v_dram = [nc.dram_tensor(f"vd{h}", [M, HD], fp32, kind="Internal")
              for h in range(HEADS)]

    with tile.TileContext(nc) as tc, \
         tc.tile_pool(name="persist", bufs=1) as pp:

        # persistent SBUF tensors
        ident = pp.tile([128, 128], fp32, tag="ident")
        nc.sync.dma_start(ident[:], ident_d[:])
        eps_t = pp.tile([128, 1], fp32, tag="eps")
        nc.gpsimd.memset(eps_t[:], EPS)

        shT = pp.tile([128, M], fp32, tag="shT")        # sh transposed [feat, rows]
        hT = pp.tile([128, ROWS], fp32, tag="hT")       # h_own transposed
        h_nat = pp.tile([128, QC * 128], fp32, tag="h_nat")  # h_own natural (per qc slices)
        kT = pp.tile([128, M], fp32, tag="kT")
        qT = pp.tile([128, ROWS], fp32, tag="qT")
        iota_b = pp.tile([128, M], fp32, tag="iota")
        nc.sync.dma_start(iota_b[:], iota_d[:])

        wt = {}
        for w in W:
            wt[w] = pp.tile([128, 128], fp32, tag=f"w_{w}", name=f"w_{w}")
            nc.sync.dma_start(wt[w][:], W[w][:])
        bt = {}
        for b in B:
            bt[b] = pp.tile([128, 1], fp32, tag=f"b_{b}", name=f"b_{b}")
            nc.sync.dma_start(bt[b][:], B[b][:])
        gt = {}
        for g in G:
            gt[g] = pp.tile([128, 128], fp32, tag=f"g_{g}", name=f"g_{g}")
            nc.sync.dma_start(gt[g][:], G[g][:])
        mst = {}
        for mname, md in [("msA", msA), ("meA", meA), ("msB", msB), ("meB", meB)]:
            mst[mname] = pp.tile([128, QC], fp32, tag=f"m_{mname}", name=f"m_{mname}")
            nc.sync.dma_start(mst[mname][:], md[:])
        bo2_t = pp.tile([128, 128], fp32, tag="bo2b")
        nc.sync.dma_start(bo2_t[:], bo2_b[:])

        # ---------------- load + transpose inputs ----------------
        with tc.tile_pool(name="ld_sb", bufs=3) as lsb, \
             tc.tile_pool(name="ld_ps", bufs=3, space="PSUM") as lps:
            for c in range(M // 128):
                t = lsb.tile([128, 128], fp32, tag="ldt")
                nc.sync.dma_start(t[:], sh[c * 128:(c + 1) * 128, :])
                ps = lps.tile([128, 128], fp32, tag="ldp")
                nc.tensor.transpose(ps[:], t[:], ident[:])
                nc.scalar.copy(shT[:, c * 128:(c + 1) * 128], ps[:])
            for c in range(QC):
                t = lsb.tile([128, 128], fp32, tag="ldt")
                nc.sync.dma_start(t[:], h_own[c * 128:(c + 1) * 128, :])
                nc.vector.tensor_copy(h_nat[:, c * 128:(c + 1) * 128], t[:])
                ps = lps.tile([128, 128], fp32, tag="ldp")
                nc.tensor.transpose(ps[:], t[:], ident[:])
                nc.scalar.copy(hT[:, c * 128:(c + 1) * 128], ps[:])

        # ---------------- MLPs (transposed activations, LN via transpose sandwich) ----------------
        def mlp_T(xT_ap, rows, w1, b1, g_b, be_b, w2, b2, outT_ap, mm_pool, tr_pool, sb_pool):
            """outT = mlp(x)^T with x given as xT [feat, rows]. All f32."""
            nchunks = rows // 512
            for c in range(nchunks):
                sl = slice(c * 512, (c + 1) * 512)
                y1p = mm_pool.tile([128, 512], fp32, tag="y1p")
                nc.tensor.matmul(y1p[:], lhsT=w1[:], rhs=xT_ap[:, sl], start=True, stop=True)
                y1s = sb_pool.tile([128, 512], fp32, tag="y1s")
                nc.vector.tensor_scalar(out=y1s[:], in0=y1p[:], scalar1=b1[:],
                                        scalar2=None, op0=AL.add)
                # transpose to natural [rows, hid]
                nat = sb_pool.tile([128, 512], fp32, tag="nat")
                for s in range(4):
                    ssl = slice(s * 128, (s + 1) * 128)
                    np_ = tr_pool.tile([128, 128], fp32, tag="trp")
                    nc.tensor.transpose(np_[:], y1s[:, ssl], ident[:])
                    nc.scalar.copy(nat[:, ssl], np_[:])
                # LN stats over hid (free dim), batched for the 4 sub-tiles
                sums = sb_pool.tile([128, 4], fp32, tag="sums")
                nc.vector.tensor_reduce(out=sums[:], in_=nat[:].rearrange("p (s f) -> p s f", f=128),
                                        axis=AX.X, op=AL.add)
                sq = sb_pool.tile([128, 512], fp32, tag="sq")
                nc.vector.tensor_tensor(out=sq[:], in0=nat[:], in1=nat[:], op=AL.mult)
                sums2 = sb_pool.tile([128, 4], fp32, tag="sums2")
                nc.vector.tensor_reduce(out=sums2[:], in_=sq[:].rearrange("p (s f) -> p s f", f=128),
                                        axis=AX.X, op=AL.add)
                mu = sb_pool.tile([128, 4], fp32, tag="mu")
                nc.vector.tensor_scalar(out=mu[:], in0=sums[:], scalar1=1.0 / 128, scalar2=None, op0=AL.mult)
                ex2 = sb_pool.tile([128, 4], fp32, tag="ex2")
                nc.vector.tensor_scalar(out=ex2[:], in0=sums2[:], scalar1=1.0 / 128, scalar2=None, op0=AL.mult)
                mu2 = sb_pool.tile([128, 4], fp32, tag="mu2")
                nc.vector.tensor_tensor(out=mu2[:], in0=mu[:], in1=mu[:], op=AL.mult)
                var = sb_pool.tile([128, 4], fp32, tag="var")
                nc.vector.tensor_tensor(out=var[:], in0=ex2[:], in1=mu2[:], op=AL.subtract)
                sd = sb_pool.tile([128, 4], fp32, tag="sd")
                nc.scalar.activation(sd[:], var[:], AF.Sqrt, bias=eps_t[:], scale=1.0)
                rstd = sb_pool.tile([128, 4], fp32, tag="rstd")
                nc.vector.reciprocal(rstd[:], sd[:])
                # normalize (+ affine) + relu, then transpose back
                nrm = sb_pool.tile([128, 512], fp32, tag="nrm")
                for s in range(4):
                    ssl = slice(s * 128, (s + 1) * 128)
                    nc.vector.tensor_scalar(out=nrm[:, ssl], in0=nat[:, ssl],
                                            scalar1=mu[:, s:s + 1], scalar2=rstd[:, s:s + 1],
                                            op0=AL.subtract, op1=AL.mult)
                if g_b is not None:
                    nc.vector.tensor_tensor(out=nrm[:], in0=nrm[:], in1=g_b[:].to_broadcast([128, 512]), op=AL.mult)
                    nc.vector.tensor_tensor(out=nrm[:], in0=nrm[:], in1=be_b[:].to_broadcast([128, 512]), op=AL.add)
                rl = sb_pool.tile([128, 512], fp32, tag="rl")
                nc.vector.tensor_scalar(out=rl[:], in0=nrm[:], scalar1=0.0, scalar2=None, op0=AL.max)
                yTr = sb_pool.tile([128, 512], fp32, tag="yTr")
                for s in range(4):
                    ssl = slice(s * 128, (s + 1) * 128)
                    np2 = tr_pool.tile([128, 128], fp32, tag="trp")
                    nc.tensor.transpose(np2[:], rl[:, ssl], ident[:])
                    nc.scalar.copy(yTr[:, ssl], np2[:])
                y2p = mm_pool.tile([128, 512], fp32, tag="y2p")
                nc.tensor.matmul(y2p[:], lhsT=w2[:], rhs=yTr[:], start=True, stop=True)
                nc.vector.tensor_scalar(out=outT_ap[:, sl], in0=y2p[:], scalar1=b2[:],
                                        scalar2=None, op0=AL.add)

        ga = (lambda k_: gt[k_] if apply_affine else None)
        with tc.tile_pool(name="mlp_mm", bufs=2, space="PSUM") as mmp, \
             tc.tile_pool(name="mlp_tr", bufs=4, space="PSUM") as trp, \
             tc.tile_pool(name="mlp_sb", bufs=2) as msb:
            mlp_T(qT_in := hT, ROWS, wt["wq1"], bt["bq1"], ga("gq_b"),
                  ga("beq_b") if apply_affine else None, wt["wq2"], bt["bq2"], qT, mmp, trp, msb)
            mlp_T(shT, M, wt["wk1"], bt["bk1"], ga("gk_b"),
                  ga("bek_b") if apply_affine else None, wt["wk2"], bt["bk2"], kT, mmp, trp, msb)
            # v: compute transposed, then transpose to natural and store per-head to DRAM
            vT = pp.tile([128, M], fp32, tag="vT")
            mlp_T(shT, M, wt["wv1"], bt["bv1"], ga("gv_b"),
                  ga("bev_b") if apply_affine else None, wt["wv2"], bt["bv2"], vT, mmp, trp, msb)
            for c in range(M // 128):
                vp = trp.tile([128, 128], fp32, tag="trp")
                nc.tensor.transpose(vp[:], vT[:, c * 128:(c + 1) * 128], ident[:])
                vn = msb.tile([128, 128], fp32, tag="vns")
                nc.scalar.copy(vn[:], vp[:])
                for h in range(HEADS):
                    nc.sync.dma_start(v_dram[h][c * 128:(c + 1) * 128, :],
                                      vn[:, h * HD:(h + 1) * HD])

        # ---------------- fp16 hi/lo splits of qT, kT ----------------
        kh16 = pp.tile([128, M], fp16, tag="kh16")
        kl16 = pp.tile([128, M], fp16, tag="kl16")
        qh16 = pp.tile([128, ROWS], fp16, tag="qh16")
        ql16 = pp.tile([128, ROWS], fp16, tag="ql16")
        with tc.tile_pool(name="split_sb", bufs=2) as ssb:
            for nm, src_t, hi, lo, width in (("k", kT, kh16, kl16, M),
                                             ("q", qT, qh16, ql16, ROWS)):
                nc.scalar.copy(hi[:], src_t[:])
                hf = ssb.tile([128, width], fp32, tag="hf", name=f"hf_{nm}")
                nc.scalar.copy(hf[:], hi[:])
                lr = ssb.tile([128, width], fp32, tag="lr", name=f"lr_{nm}")
                nc.vector.tensor_tensor(out=lr[:], in0=src_t[:], in1=hf[:], op=AL.subtract)
                nc.scalar.copy(lo[:], lr[:])

        # ---------------- attention: scores + masked argmin + gather ----------------
        attn = pp.tile([128, QC * 128], fp32, tag="attn")   # gathered v rows, natural layout
        with tc.tile_pool(name="z_ps", bufs=2, space="PSUM") as zp, \
             tc.tile_pool(name="att_sb", bufs=2) as asb, \
             tc.tile_pool(name="att_small", bufs=4) as ats:
            for h in range(HEADS):
                hsl = slice(h * HD, (h + 1) * HD)
                # stage this head's fp16 pieces at partition base 0:
                # rows [0:8]=hi, [8:16]=hi(q)/lo(k), [16:24]=lo(q)/hi(k)
                # pairing: qh*kh + qh*kl + ql*kh
                q_st = ats.tile([3 * HD, ROWS], fp16, tag="q_st")
                nc.sync.dma_start(q_st[0 * HD:1 * HD, :], qh16[hsl, :])
                nc.sync.dma_start(q_st[1 * HD:2 * HD, :], qh16[hsl, :])
                nc.sync.dma_start(q_st[2 * HD:3 * HD, :], ql16[hsl, :])
                k_st = asb.tile([3 * HD, M], fp16, tag="k_st")
                nc.sync.dma_start(k_st[0 * HD:1 * HD, :], kh16[hsl, :])
                nc.sync.dma_start(k_st[1 * HD:2 * HD, :], kl16[hsl, :])
                nc.sync.dma_start(k_st[2 * HD:3 * HD, :], kh16[hsl, :])
                for qc in range(QC):
                    qsl = slice(qc * 128, (qc + 1) * 128)
                    Wm = asb.tile([128, M], fp32, tag="Wm")
                    racc = None
                    for half in range(2):
                        zt = zp.tile([128, 2048], fp32, tag="zt")
                        for j in range(4):
                            col0 = half * 2048 + j * 512
                            nc.tensor.matmul(zt[:, j * 512:(j + 1) * 512],
                                             lhsT=q_st[:, qsl],
                                             rhs=k_st[:, col0:col0 + 512],
                                             start=True, stop=True)
                        rnew = ats.tile([128, 1], fp32, tag="racc")
                        ms = mst["msA" if half == 0 else "msB"]
                        me = mst["meA" if half == 0 else "meB"]
                        nc.vector._custom_dve(
                            TENSOR_MASK_REDUCE,
                            out=Wm[:, half * 2048:(half + 1) * 2048], in0=zt[:],
                            in1=me[:, qc:qc + 1],
                            s0=ms[:, qc:qc + 1],
                            s1=(NEG_BIG if racc is None else racc[:]),
                            imm2=1.0, accum_out=rnew[:])
                        racc = rnew
                    # index of the (negated-score) maximum. Indicator spike on
                    # the idle ScalarE: E = relu(Wm*S + (1 - R*S)), S=1e6 --
                    # fires ~1.0 at the max (ACT applies scale/bias as one
                    # fma). accum_out gives the normalizer for free.
                    SPK = 1.0e6
                    ebias = ats.tile([128, 1], fp32, tag="ebias")
                    nc.vector.tensor_scalar(out=ebias[:], in0=racc[:],
                                            scalar1=-SPK, scalar2=1.0,
                                            op0=AL.mult, op1=AL.add)
                    E = asb.tile([128, M], fp32, tag="E")
                    cnt = ats.tile([128, 1], fp32, tag="cnt")
                    nc.scalar.activation(E[:], Wm[:], AF.Relu,
                                         bias=ebias[:], scale=SPK,
                                         accum_out=cnt[:])
                    idxf = ats.tile([128, 1], fp32, tag="idxf")
                    nc.vector._custom_dve(TENSOR_TENSOR_REDUCE,
                                          out=E[:], in0=E[:], in1=iota_b[:],
                                          s0=0.0, s1=1.0, accum_out=idxf[:])
                    rcnt = ats.tile([128, 1], fp32, tag="rcnt")
                    nc.vector.reciprocal(rcnt[:], cnt[:])
                    idxn = ats.tile([128, 1], fp32, tag="idxn")
                    nc.vector.tensor_scalar(out=idxn[:], in0=idxf[:],
                                            scalar1=rcnt[:], scalar2=4095.0,
                                            op0=AL.mult, op1=AL.min)
                    idxi = ats.tile([128, 1], int32, tag="idxi")
                    nc.vector.tensor_copy(idxi[:], idxn[:])
                    if gather_mode == "indirect":
                        nc.gpsimd.indirect_dma_start(
                            out=attn[:, qc * 128 + h * HD: qc * 128 + (h + 1) * HD],
                            out_offset=None,
                            in_=v_dram[h][:],
                            in_offset=bass.IndirectOffsetOnAxis(ap=idxi[:, :1], axis=0),
                        )
                    else:
                        nc.sync.dma_start(
                            out=attn[:, qc * 128 + h * HD: qc * 128 + (h + 1) * HD],
                            in_=v_dram[h][0:128, :])

        # ---------------- output MLP + residual ----------------
        with tc.tile_pool(name="o_mm", bufs=2, space="PSUM") as omp, \
             tc.tile_pool(name="o_tr", bufs=4, space="PSUM") as otp, \
             tc.tile_pool(name="o_sb", bufs=3) as osb:
            # attn^T
            attnT = pp.tile([128, ROWS], fp32, tag="attnT")
            for c in range(QC):
                ap_ = otp.tile([128, 128], fp32, tag="atp")
                nc.tensor.transpose(ap_[:], attn[:, c * 128:(c + 1) * 128], ident[:])
                nc.scalar.copy(attnT[:, c * 128:(c + 1) * 128], ap_[:])
            # layer1: y1 = cat(attn, h) @ Wo1 (+bo1)  in transposed layout
            y1p = omp.tile([128, 512], fp32, tag="oy1p")
            nc.tensor.matmul(y1p[:], lhsT=wt["wo1a"][:], rhs=attnT[:], start=True, stop=False)
            nc.tensor.matmul(y1p[:], lhsT=wt["wo1b"][:], rhs=hT[:], start=False, stop=True)
            y1s = osb.tile([128, 512], fp32, tag="oy1s")
            nc.vector.tensor_scalar(out=y1s[:], in0=y1p[:], scalar1=bt["bo1"][:],
                                    scalar2=None, op0=AL.add)
            nat = osb.tile([128, 512], fp32, tag="onat")
            for s in range(4):
                ssl = slice(s * 128, (s + 1) * 128)
                np_ = otp.tile([128, 128], fp32, tag="atp")
                nc.tensor.transpose(np_[:], y1s[:, ssl], ident[:])
                nc.scalar.copy(nat[:, ssl], np_[:])
            sums = osb.tile([128, 4], fp32, tag="osums")
            nc.vector.tensor_reduce(out=sums[:], in_=nat[:].rearrange("p (s f) -> p s f", f=128),
                                    axis=AX.X, op=AL.add)
            sq = osb.tile([128, 512], fp32, tag="osq")
            nc.vector.tensor_tensor(out=sq[:], in0=nat[:], in1=nat[:], op=AL.mult)
            sums2 = osb.tile([128, 4], fp32, tag="osums2")
            nc.vector.tensor_reduce(out=sums2[:], in_=sq[:].rearrange("p (s f) -> p s f", f=128),
                                    axis=AX.X, op=AL.add)
            mu = osb.tile([128, 4], fp32, tag="omu")
            nc.vector.tensor_scalar(out=mu[:], in0=sums[:], scalar1=1.0 / 128, scalar2=None, op0=AL.mult)
            ex2 = osb.tile([128, 4], fp32, tag="oex2")
            nc.vector.tensor_scalar(out=ex2[:], in0=sums2[:], scalar1=1.0 / 128, scalar2=None, op0=AL.mult)
            mu2 = osb.tile([128, 4], fp32, tag="omu2")
            nc.vector.tensor_tensor(out=mu2[:], in0=mu[:], in1=mu[:], op=AL.mult)
            var = osb.tile([128, 4], fp32, tag="ovar")
            nc.vector.tensor_tensor(out=var[:], in0=ex2[:], in1=mu2[:], op=AL.subtract)
            sd = osb.tile([128, 4], fp32, tag="osd")
            nc.scalar.activation(sd[:], var[:], AF.Sqrt, bias=eps_t[:], scale=1.0)
            rstd = osb.tile([128, 4], fp32, tag="orstd")
            nc.vector.reciprocal(rstd[:], sd[:])
            nrm = osb.tile([128, 512], fp32, tag="onrm")
            for s in range(4):
                ssl = slice(s * 128, (s + 1) * 128)
                nc.vector.tensor_scalar(out=nrm[:, ssl], in0=nat[:, ssl],
                                        scalar1=mu[:, s:s + 1], scalar2=rstd[:, s:s + 1],
                                        op0=AL.subtract, op1=AL.mult)
            if apply_affine:
                nc.vector.tensor_tensor(out=nrm[:], in0=nrm[:], in1=gt["go_b"][:].to_broadcast([128, 512]), op=AL.mult)
                nc.vector.tensor_tensor(out=nrm[:], in0=nrm[:], in1=gt["beo_b"][:].to_broadcast([128, 512]), op=AL.add)
            rl = osb.tile([128, 512], fp32, tag="orl")
            nc.vector.tensor_scalar(out=rl[:], in0=nrm[:], scalar1=0.0, scalar2=None, op0=AL.max)
            # layer2 in natural layout per qc chunk + bias + residual
            for s in range(4):
                ssl = slice(s * 128, (s + 1) * 128)
                yTr = osb.tile([128, 128], fp32, tag="oyTr")
                np2 = otp.tile([128, 128], fp32, tag="atp")
                nc.tensor.transpose(np2[:], rl[:, ssl], ident[:])
                nc.scalar.copy(yTr[:], np2[:])
                y2p = omp.tile([128, 128], fp32, tag="oy2p")
                nc.tensor.matmul(y2p[:], lhsT=yTr[:], rhs=wt["wo2"][:], start=True, stop=True)
                fin = osb.tile([128, 128], fp32, tag="fin")
                # + bo2 (replicated across partitions on host; free dim = out feature)
                nc.vector.tensor_tensor(out=fin[:], in0=y2p[:], in1=bo2_t[:], op=AL.add)
                nc.vector.tensor_tensor(out=fin[:], in0=fin[:], in1=h_nat[:, ssl], op=AL.add)
                nc.sync.dma_start(out_d[s * 128:(s + 1) * 128, :], fin[:])

    nc.compile()
    return nc


def _window_encoding(batch, sub_batch):
    """Per-row (mask_start, mask_end) for the two 2048-wide halves."""
    b = np.asarray(batch).astype(np.int64)
    sb = np.asarray(sub_batch).astype(np.int64)
    a = np.searchsorted(sb, b, side="left").astype(np.int64)
    e = np.searchsorted(sb, b, side="right").astype(np.int64)
    enc = []
    for off in (0, 2048):
        wa = np.clip(a - off, 0, 2048)
        wb = np.clip(e - off, 0, 2048)
        ms = np.where(wa == wb, 0.0, wb.astype(np.float64))
        me = np.where(wa == wb, 2048.0, wa.astype(np.float64))
        enc.append((ms.astype(np.float32), me.astype(np.float32)))
    return enc  # [(msA, meA), (msB, meB)] each [N]


def kernel(h, sh, batch, sub_batch, params):
    from concourse.bass_utils import run_bass_kernel_spmd

    h = np.ascontiguousarray(np.asarray(h, dtype=np.float32))
    sh = np.ascontiguousarray(np.asarray(sh, dtype=np.float32))
    p = params

    def col(x):
        return np.ascontiguousarray(np.asarray(x, dtype=np.float32).reshape(D, 1))

    def matf(x):
        return np.ascontiguousarray(np.asarray(x, dtype=np.float32))

    affine_identity = all(
        np.all(np.asarray(p[k]["b1"]) == 0) and np.all(np.asarray(p[k]["g"]) == 1)
        and np.all(np.asarray(p[k]["be"]) == 0)
        for k in ("hk", "hv", "hq", "out")
    )
    apply_affine = not affine_identity

    import os
    gather_mode = os.environ.get("GATHER_MODE", "indirect")
    key = ("prog", apply_affine, gather_mode)
    if key not in _CACHED:
        _CACHED[key] = _build_program(apply_affine, gather_mode)
    nc = _CACHED[key]

    (msA, meA), (msB, meB) = _window_encoding(batch, sub_batch)
    iota_b = np.ascontiguousarray(
        np.broadcast_to(np.arange(M, dtype=np.float32), (D, M)))

    base = {
        "sh": sh,
        "wq1": matf(p["hq"]["w1"]), "wq2": matf(-np.asarray(p["hq"]["w2"], dtype=np.float32)),
        "wk1": matf(p["hk"]["w1"]), "wk2": matf(p["hk"]["w2"]),
        "wv1": matf(p["hv"]["w1"]), "wv2": matf(p["hv"]["w2"]),
        "wo1a": matf(np.asarray(p["out"]["w1"], dtype=np.float32)[:D, :]),
        "wo1b": matf(np.asarray(p["out"]["w1"], dtype=np.float32)[D:, :]),
        "wo2": matf(p["out"]["w2"]),
        "bq1": col(p["hq"]["b1"]), "bq2": col(-np.asarray(p["hq"]["b2"], dtype=np.float32)),
        "bk1": col(p["hk"]["b1"]), "bk2": col(p["hk"]["b2"]),
        "bv1": col(p["hv"]["b1"]), "bv2": col(p["hv"]["b2"]),
        "bo1": col(p["out"]["b1"]),
        "bo2_b": np.ascontiguousarray(np.broadcast_to(
            np.asarray(p["out"]["b2"], dtype=np.float32)[None, :], (D, D))),
        "iota_b": iota_b,
        "ident": np.eye(D, dtype=np.float32),
    }
    if apply_affine:
        def bcast(x):
            return np.ascontiguousarray(
                np.broadcast_to(np.asarray(x, dtype=np.float32)[None, :], (D, D)))
        base.update({
            "gq_b": bcast(p["hq"]["g"]), "beq_b": bcast(p["hq"]["be"]),
            "gk_b": bcast(p["hk"]["g"]), "bek_b": bcast(p["hk"]["be"]),
            "gv_b": bcast(p["hv"]["g"]), "bev_b": bcast(p["hv"]["be"]),
            "go_b": bcast(p["out"]["g"]), "beo_b": bcast(p["out"]["be"]),
        })

    def chunkify(x, c):
        # [512] row-vector -> [128, QC] column-per-qc layout
        return np.ascontiguousarray(
            x[c * ROWS:(c + 1) * ROWS].reshape(QC, 128).T.astype(np.float32))

    in_maps = []
    for c in range(NCORES):
        m = dict(base)
        m["h_own"] = np.ascontiguousarray(h[c * ROWS:(c + 1) * ROWS, :])
        m["msA"] = chunkify(msA, c)
        m["meA"] = chunkify(meA, c)
        m["msB"] = chunkify(msB, c)
        m["meB"] = chunkify(meB, c)
        in_maps.append(m)

    global _last_in_maps
    _last_in_maps = in_maps
    res = run_bass_kernel_spmd(nc, in_maps, core_ids=list(range(NCORES)))
    out = np.concatenate([res.results[c]["out"] for c in range(NCORES)], axis=0)
    return out.astype(np.float32)


if __name__ == "__main__":
    import reference
    inp = reference.setup_inputs()
    exp = np.asarray(reference.reference(**inp))
    act = kernel(**{k: (v if isinstance(v, dict) else np.asarray(v)) for k, v in inp.items()})
    err = np.linalg.norm(act - exp) / np.linalg.norm(exp)
    print("Relative error:", err)
